# revision 8
# baseline (speedup 1.0000x reference)
"""Trainium2 Bass kernel for 16-head MultiHeadAttention (B=4, S=2048, D=1024).

Sharding: 8 cores = 4 batches x 2 head-groups (Megatron-style tensor
parallelism inside a batch).  Core c handles batch c//2 and heads
(c%2)*8 .. +8.  Q/K/V projection weights are column-sharded, Wo is
row-sharded; the 2-way partial sum of the output projection plus the bo
bias is applied on the host after gathering.

Device layout choices (per core):
  - Activations arrive host-pre-transposed: X^T [1024, 2048].
  - Q and K are produced directly in transposed layout QT/KT [d, s]
    (head dim on partitions), V in natural layout [s, d] with a ones
    column appended per head (so the attention row-sum rides along the
    ctx matmul as output row 64).
  - Attention runs in transposed orientation: logitsT [sk, sq] =
    (K Q^T), so softmax's additive mask is a per-partition ACT bias and
    exp needs no max-subtraction (logits are O(1) by construction).
  - ctx^T [d, sq] = V^T @ attnT accumulates over sk tiles in PSUM;
    row 64 is the softmax denominator.  Normalization multiplies by the
    broadcast reciprocal before the output projection.
"""

import os
import sys

for _p in ("/opt/trn_rl_repo", "/root/.axon_site/_ro/trn_rl_repo"):
    if os.path.isdir(_p) and _p not in sys.path:
        sys.path.insert(0, _p)

import numpy as np

import concourse.bass as bass
import concourse.mybir as mybir
import concourse.tile as tile
from concourse import bacc
from concourse.bass_utils import run_bass_kernel_spmd

# ---------------------------------------------------------------- constants
B = 4
S = 2048
D = 1024
NH = 16          # total heads
DK = 64          # head dim
N_CORES = 8
H = 8            # heads per core
DH = H * DK      # 512 local d_model columns
KT_N = D // 128  # 8 contraction k-tiles
NDT = DH // 128  # 4 d-tiles of QT/KT
NSC = S // 512   # 4 s-chunks
NST = S // 128   # 16 s-tiles
VSTRIDE = H * (DK + 1)  # 520: v tile columns per s-tile (ones col per head)
ECH = 512
NECH = D // ECH  # 2 output column chunks
SCALE = 1.0 / np.sqrt(np.float32(DK))

F32 = mybir.dt.float32
F32R = mybir.dt.float32r

BF16 = mybir.dt.bfloat16

# matmul input dtype mode: "f32" (exact, 4 cyc/row), "f32r" (1 cyc/row),
# or "bf16" (1 cyc/row, half SBUF/DMA, separate overlapped weight loads)
MM_MODE = os.environ.get("MHA_MM_MODE", "f32r")
MMDT = {"f32": F32, "f32r": F32R, "bf16": BF16}[MM_MODE]
NPDT = np.float32 if MM_MODE != "bf16" else None  # host array dtype, see prep

_CACHE = {}


def build_kernel(has_bias: bool):
    nc = bacc.Bacc(
        "TRN2",
        target_bir_lowering=False,
        debug=False,
        num_devices=N_CORES,
        dynamic_dma_scratch_size=2048,
    )

    KR = D + (1 if has_bias else 0)  # input rows incl. optional bias row
    xq = nc.dram_tensor("xq", (KR, S), MMDT, kind="ExternalInput")
    xk = nc.dram_tensor("xk", (KR, S), MMDT, kind="ExternalInput")
    xv = nc.dram_tensor("xv", (KR, S), MMDT, kind="ExternalInput")
    wq = nc.dram_tensor("wq", (KR, DH), MMDT, kind="ExternalInput")
    wk = nc.dram_tensor("wk", (KR, DH), MMDT, kind="ExternalInput")
    wv = nc.dram_tensor("wv", (KR, DH), MMDT, kind="ExternalInput")
    wo = nc.dram_tensor("wo", (DH, D), MMDT, kind="ExternalInput")
    mb = nc.dram_tensor("mb", (128, NST), F32, kind="ExternalInput")
    ones_d = nc.dram_tensor("ones", (128, NST * H), MMDT, kind="ExternalInput")
    out = nc.dram_tensor("out", (S, D), F32, kind="ExternalOutput")

    with tile.TileContext(nc) as tc:
        with tc.tile_pool(name="persist", bufs=1) as pp:
            qt_sb = pp.tile([128, NDT * S], MMDT, tag="qt")    # [d-tile | s]
            kt_sb = pp.tile([128, NDT * S], MMDT, tag="kt")
            v_sb = pp.tile([128, NST * VSTRIDE], MMDT, tag="v")
            mb_sb = pp.tile([128, NST], F32, tag="mb")

            nc.sync.dma_start(mb_sb[:], mb[:, :])
            # ones columns for the rowsum trick (DMA keeps dtype consistent)
            for st in range(NST):
                nc.sync.dma_start(
                    v_sb[:, st * VSTRIDE + DK:st * VSTRIDE + VSTRIDE:DK + 1],
                    ones_d[:, st * H:(st + 1) * H],
                )

            # ---------------------------------------------- projections
            with (
                tc.tile_pool(name="xpool", bufs=1) as xp,
                tc.tile_pool(name="wpool", bufs=2) as wp,
                tc.tile_pool(name="augp", bufs=2) as ap_,
                tc.tile_pool(name="ppsum", bufs=6, space="PSUM") as pps,
            ):
                for which, xd, wd in (("q", xq, wq), ("k", xk, wk), ("v", xv, wv)):
                    w_sb = wp.tile([128, KT_N * DH], MMDT, tag="w")
                    for kt in range(KT_N):
                        nc.sync.dma_start(
                            w_sb[:, kt * DH:(kt + 1) * DH],
                            wd[kt * 128:(kt + 1) * 128, :],
                        )
                    if has_bias:
                        w_aug = ap_.tile([1, DH], MMDT, tag="waug")
                        nc.sync.dma_start(w_aug[:], wd[D:D + 1, :])

                    for half in range(2):
                        s0 = half * 1024
                        x_sb = xp.tile([128, KT_N * 1024], MMDT, tag="x")
                        for kt in range(KT_N):
                            nc.sync.dma_start(
                                x_sb[:, kt * 1024:(kt + 1) * 1024],
                                xd[kt * 128:(kt + 1) * 128, s0:s0 + 1024],
                            )
                        if has_bias:
                            x_aug = ap_.tile([1, 1024], MMDT, tag="xaug")
                            nc.sync.dma_start(x_aug[:], xd[D:D + 1, s0:s0 + 1024])

                        if which in ("q", "k"):
                            dst = qt_sb if which == "q" else kt_sb
                            for dt in range(NDT):
                                ps = [pps.tile([128, 512], F32, tag="proj", name=f"ps{j2}")
                                      for j2 in range(2)]
                                for kt in range(KT_N):
                                    for j2 in range(2):
                                        nc.tensor.matmul(
                                            ps[j2][:],
                                            (w_sb[:, kt * DH + dt * 128:
                                                     kt * DH + dt * 128 + 128]),
                                            (x_sb[:, kt * 1024 + j2 * 512:
                                                     kt * 1024 + j2 * 512 + 512]),
                                            start=(kt == 0),
                                            stop=(kt == KT_N - 1 and not has_bias),
                                        )
                                if has_bias:
                                    for j2 in range(2):
                                        nc.tensor.matmul(
                                            ps[j2][:],
                                            (w_aug[0:1, dt * 128:dt * 128 + 128]),
                                            (x_aug[0:1, j2 * 512:j2 * 512 + 512]),
                                            start=False, stop=True,
                                        )
                                for j2 in range(2):
                                    nc.vector.tensor_copy(
                                        dst[:, dt * S + s0 + j2 * 512:
                                            dt * S + s0 + j2 * 512 + 512],
                                        ps[j2][:],
                                    )
                        else:  # V: natural layout per s-tile, per-head cols
                            for st4 in range(8):
                                st = half * 8 + st4
                                psv = pps.tile([128, 512], F32, tag="proj")
                                for kt in range(KT_N):
                                    nc.tensor.matmul(
                                        psv[:],
                                        (x_sb[:, kt * 1024 + st4 * 128:
                                                 kt * 1024 + st4 * 128 + 128]),
                                        (w_sb[:, kt * DH:(kt + 1) * DH]),
                                        start=(kt == 0),
                                        stop=(kt == KT_N - 1 and not has_bias),
                                    )
                                if has_bias:
                                    nc.tensor.matmul(
                                        psv[:],
                                        (x_aug[0:1, st4 * 128:st4 * 128 + 128]),
                                        (w_aug[0:1, :]),
                                        start=False, stop=True,
                                    )
                                for h in range(H):
                                    nc.vector.tensor_copy(
                                        v_sb[:, st * VSTRIDE + h * (DK + 1):
                                             st * VSTRIDE + h * (DK + 1) + DK],
                                        psv[:, h * DK:(h + 1) * DK],
                                    )

            # ---------------------------------------------- attention + out-proj
            with (
                tc.tile_pool(name="wop", bufs=1) as wop,
                tc.tile_pool(name="atp", bufs=4) as atp,
                tc.tile_pool(name="nrm", bufs=2) as nrm,
                tc.tile_pool(name="ctxn", bufs=2) as cxp,
                tc.tile_pool(name="obp", bufs=3) as obp,
                tc.tile_pool(name="bigps", bufs=4, space="PSUM") as bps,
                tc.tile_pool(name="ctxps", bufs=4, space="PSUM") as cps,
            ):
                wo_sb = wop.tile([64, H * D], MMDT, tag="wo")
                for h in range(H):
                    nc.sync.dma_start(
                        wo_sb[0:64, h * D:(h + 1) * D],
                        wo[h * DK:(h + 1) * DK, :],
                    )

                for j in range(NSC):
                    ctxn = cxp.tile([64, H * 512], MMDT, tag="ctxn")
                    for hp in range(NDT):
                        heads = (2 * hp, 2 * hp + 1)
                        ctx_ps = {h: cps.tile([65, 512], F32, tag="ctx", name=f"ctx{h}")
                                  for h in heads}
                        for i in range(NST):
                            for h in heads:
                                pb = (h % 2) * 64
                                lg = bps.tile([128, 512], F32, tag="big")
                                nc.tensor.matmul(
                                    lg[:],
                                    (kt_sb[pb:pb + 64,
                                              hp * S + i * 128:hp * S + i * 128 + 128]),
                                    (qt_sb[pb:pb + 64,
                                              hp * S + j * 512:hp * S + j * 512 + 512]),
                                    start=True, stop=True,
                                )
                                at = atp.tile([128, 512], MMDT, tag="at")
                                nc.scalar.activation(
                                    at[:], lg[:],
                                    mybir.ActivationFunctionType.Exp,
                                    bias=mb_sb[:, i:i + 1],
                                    scale=float(SCALE),
                                )
                                nc.tensor.matmul(
                                    ctx_ps[h][:],
                                    (v_sb[:, i * VSTRIDE + h * (DK + 1):
                                             i * VSTRIDE + h * (DK + 1) + DK + 1]),
                                    (at[:]),
                                    start=(i == 0), stop=(i == NST - 1),
                                )
                        for h in heads:
                            stg = nrm.tile([65, 512], F32, tag="stg")
                            nc.vector.tensor_copy(stg[64:65, :], ctx_ps[h][64:65, :])
                            rs0 = nrm.tile([1, 512], F32, tag="rs0")
                            nc.sync.dma_start(rs0[:], stg[64:65, :])
                            rsr = nrm.tile([1, 512], F32, tag="rsr")
                            nc.vector.reciprocal(rsr[:], rs0[:])
                            bc = nrm.tile([64, 512], F32, tag="bc")
                            nc.gpsimd.partition_broadcast(bc[:], rsr[0:1, :])
                            nc.vector.tensor_mul(
                                ctxn[:, h * 512:(h + 1) * 512],
                                ctx_ps[h][0:64, :], bc[:],
                            )
                    # output projection for s-chunk j
                    for t in range(4):
                        sq = j * 512 + t * 128
                        for ec in range(NECH):
                            po = bps.tile([128, 512], F32, tag="big")
                            for h in range(H):
                                nc.tensor.matmul(
                                    po[:],
                                    (ctxn[0:64, h * 512 + t * 128:
                                             h * 512 + t * 128 + 128]),
                                    (wo_sb[0:64, h * D + ec * ECH:
                                              h * D + ec * ECH + ECH]),
                                    start=(h == 0), stop=(h == H - 1),
                                )
                            ob = obp.tile([128, 512], F32, tag="ob")
                            nc.vector.tensor_copy(ob[:], po[:])
                            nc.sync.dma_start(
                                out[sq:sq + 128, ec * ECH:ec * ECH + ECH],
                                ob[:],
                            )

    nc.compile()
    return nc


def _get_kernel(has_bias: bool):
    key = (has_bias, MM_MODE)
    if key not in _CACHE:
        _CACHE[key] = build_kernel(has_bias)
    return _CACHE[key]


def prep_in_maps(query, key, value, mask, Wq, bq, Wk, bk, Wv, bv, Wo, bo):
    query = np.asarray(query, dtype=np.float32)
    key = np.asarray(key, dtype=np.float32)
    value = np.asarray(value, dtype=np.float32)
    mask = np.asarray(mask, dtype=np.float32)
    Wq = np.asarray(Wq, dtype=np.float32)
    Wk = np.asarray(Wk, dtype=np.float32)
    Wv = np.asarray(Wv, dtype=np.float32)
    Wo = np.asarray(Wo, dtype=np.float32)
    bq = np.asarray(bq, dtype=np.float32)
    bk = np.asarray(bk, dtype=np.float32)
    bv = np.asarray(bv, dtype=np.float32)
    bo = np.asarray(bo, dtype=np.float32)

    has_bias = bool(np.any(bq) or np.any(bk) or np.any(bv))

    ones_row = np.ones((1, S), dtype=np.float32)
    in_maps = []
    for c in range(N_CORES):
        b, g = divmod(c, 2)
        cols = slice(g * DH, (g + 1) * DH)
        xq_c = np.ascontiguousarray(query[b].T)
        xk_c = np.ascontiguousarray(key[b].T)
        xv_c = np.ascontiguousarray(value[b].T)
        wq_c = np.ascontiguousarray(Wq[:, cols])
        wk_c = np.ascontiguousarray(Wk[:, cols])
        wv_c = np.ascontiguousarray(Wv[:, cols])
        if has_bias:
            xq_c = np.concatenate([xq_c, ones_row], axis=0)
            xk_c = np.concatenate([xk_c, ones_row], axis=0)
            xv_c = np.concatenate([xv_c, ones_row], axis=0)
            wq_c = np.concatenate([wq_c, bq[None, cols]], axis=0)
            wk_c = np.concatenate([wk_c, bk[None, cols]], axis=0)
            wv_c = np.concatenate([wv_c, bv[None, cols]], axis=0)
        mb_c = np.ascontiguousarray(
            mask[b, 0, 0].reshape(NST, 128).T * np.float32(-1e9))
        in_maps.append({
            "xq": xq_c, "xk": xk_c, "xv": xv_c,
            "wq": wq_c, "wk": wk_c, "wv": wv_c,
            "wo": np.ascontiguousarray(Wo[cols, :]),
            "mb": mb_c,
            "ones": np.ones((128, NST * H), dtype=np.float32),
        })
    if MM_MODE == "bf16":
        import ml_dtypes
        for m in in_maps:
            for k in m:
                if k != "mb":
                    m[k] = m[k].astype(ml_dtypes.bfloat16)
    return has_bias, in_maps, bo


def assemble_out(per_core_out, bo):
    out = np.empty((B, S, D), dtype=np.float32)
    for b in range(B):
        out[b] = per_core_out[2 * b] + per_core_out[2 * b + 1] + bo
    return out


def kernel(**inputs):
    has_bias, in_maps, bo = prep_in_maps(**inputs)
    nc = _get_kernel(has_bias)
    res = run_bass_kernel_spmd(nc, in_maps, core_ids=list(range(N_CORES)))
    return assemble_out([res.results[c]["out"] for c in range(N_CORES)], bo)


if __name__ == "__main__":
    rng = np.random.default_rng(0)
    q = rng.standard_normal((B, S, D)).astype(np.float32)
    k = rng.standard_normal((B, S, D)).astype(np.float32)
    v = rng.standard_normal((B, S, D)).astype(np.float32)
    m = np.zeros((B, 1, 1, S), dtype=np.float32)
    sc = 1.0 / np.sqrt(D)
    Wq = (rng.standard_normal((D, D)) * sc).astype(np.float32)
    Wk = (rng.standard_normal((D, D)) * sc).astype(np.float32)
    Wv = (rng.standard_normal((D, D)) * sc).astype(np.float32)
    Wo = (rng.standard_normal((D, D)) * sc).astype(np.float32)
    z = np.zeros(D, dtype=np.float32)
    o = kernel(q, k, v, m, Wq, z, Wk, z, Wv, z, Wo, z)
    print("out", o.shape, o.dtype, float(np.abs(o).mean()))
